# revision 1
# baseline (speedup 1.0000x reference)
# Trainium2 Bass kernel for: ConvTranspose2d(64->128, k=4, stride=1) -> spatial
# mean -> +biases -> 10*logsumexp over channels.
#
# Math: with full (K-1) output padding, the mean over the ENTIRE conv-transpose
# output spatial extent sees every input pixel through all K*K taps, so
#   pooled[n,co] = (sum_hw x[n,ci,hw]) @ (sum_kk w[ci,co,kk]) / (Ho*Wo) + cb + eb
# exactly. The conv collapses to a spatial sum + a (Cin x Cout) matmul.
#
# Sharding: data-parallel over batch N=32 across 8 cores (4 batches/core),
# params replicated. Each core streams its 4 MiB x-slice (8 chunked HWDGE DMAs
# on the SP ring), reduces spatially on DVE, runs two tiny accumulating PE
# matmuls (bias + data), then exp-accumulate + log on ACT.
#
# Layout trick: the spatial sums land as s2[p, r] with p = (n%2)*64 + ci,
# r = n//2 (two 128-row blocks of x). Instead of shuffling partitions, the
# weight-sum matrix is duplicated onto both partition halves (tiny SBUF->SBUF
# DMA) and s2 is expanded into a zero-masked (128, 4) lhsT so a single
# 128-deep matmul contracts each batch against its own partition half.
#
# Trace-driven choices (see test.py profiling):
# - weight/bias DMAs ride the ACT HWDGE ring, parallel to x on the SP ring;
#   a SWDGE broadcast DMA would split by the dup dim onto 2 SDMA engines and
#   starve the x stream.
# - all spatial reduces stay on DVE: mixing in ACT Copy+accum ops evicts the
#   Exp/Ln activation tables and puts two 1.3us ACT_TABLE_LOADs on the tail.
# - one pre-placed LoadActFuncSet covering BOTH Exp and Ln runs at kernel
#   start, so no ACT_TABLE_LOAD lands on the critical tail.

import os

import numpy as np

import concourse.bacc as bacc
import concourse.bass as bass
import concourse.mybir as mybir
import concourse.tile as tile
from concourse.bass_utils import run_bass_kernel_spmd
from concourse.hw_specs import get_activation_tables

N, CIN, COUT, K, H, W = 32, 64, 128, 4, 64, 64
NCORES = 8
NLOC = N // NCORES          # 4 batches per core
HW = H * W                  # 4096
ROWS = NLOC * CIN           # 256 rows (n,ci) per core
RBLK = ROWS // 128          # 2 row blocks of 128 partitions
CHUNK = 1024                # column chunk width (512 KiB per DMA)
NCHUNK = HW // CHUNK        # 4
SCALE = 1.0 / float((H + K - 1) * (W + K - 1))   # 1/4489

F32 = mybir.dt.float32

_CACHE: dict = {}


def _build_module() -> bacc.Bacc:
    nc = bacc.Bacc("TRN2", target_bir_lowering=False, enable_partition_id=False)

    x_d = nc.dram_tensor("xc", [ROWS, HW], F32, kind="ExternalInput").ap()
    w_d = nc.dram_tensor("w", [CIN, COUT * K * K], F32, kind="ExternalInput").ap()
    bs_d = nc.dram_tensor("bs", [2, COUT], F32, kind="ExternalInput").ap()
    y_d = nc.dram_tensor("y", [NLOC, 1], F32, kind="ExternalOutput").ap()

    with tile.TileContext(nc) as tc:
        with (
            tc.tile_pool(name="xpool", bufs=RBLK * NCHUNK) as xpool,
            tc.tile_pool(name="small", bufs=1) as small,
            tc.tile_pool(name="psum", bufs=1, space="PSUM") as psum_pool,
        ):
            # preload the one ACT table set that covers BOTH Exp and Ln
            # ("natural_log_exp_and_others"), so insert_act_table_loads has
            # nothing to add and no 1.3us ACT_TABLE_LOAD lands on the tail
            # between exp and ln (the pass's per-use set choice would pick
            # two different sets and reload mid-chain otherwise).
            act_tables = get_activation_tables(nc.m.arch)
            set_id = next(
                i
                for i, (_, funcs) in enumerate(act_tables.items())
                if mybir.ActivationFunctionType.Exp in funcs
                and mybir.ActivationFunctionType.Ln in funcs
            )
            nc.scalar.add_instruction(
                mybir.InstLoadActFuncSet(
                    name=nc.get_next_instruction_name(), act_func_set_id=set_id
                )
            )

            # ---- params ----
            wk = small.tile([CIN, COUT * K * K], F32)
            nc.scalar.dma_start(out=wk, in_=w_d)
            wdup = small.tile([128, COUT], F32)
            nc.vector.reduce_sum(
                out=wdup[0:CIN, :],
                in_=wk.rearrange("p (c k) -> p c k", k=K * K),
                axis=mybir.AxisListType.X,
            )
            # fold the 1/(Ho*Wo) mean scale into the weight sums
            nc.vector.tensor_scalar_mul(
                out=wdup[0:CIN, :], in0=wdup[0:CIN, :], scalar1=SCALE
            )
            # duplicate onto the other partition half (32 KB on-chip copy)
            nc.scalar.dma_start(out=wdup[CIN:128, :], in_=wdup[0:CIN, :])

            biasrows = small.tile([2, COUT], F32)
            nc.scalar.dma_start(out=biasrows, in_=bs_d)
            onesb = small.tile([2, NLOC], F32)
            nc.vector.memset(onesb, 1.0)

            # ---- spatial sums of x (all on DVE) ----
            parts = small.tile([128, RBLK * NCHUNK], F32)
            for r in range(RBLK):
                for c in range(NCHUNK):
                    xt = xpool.tile([128, CHUNK], F32, tag="xt")
                    nc.sync.dma_start(
                        out=xt,
                        in_=x_d[r * 128 : (r + 1) * 128, c * CHUNK : (c + 1) * CHUNK],
                    )
                    nc.vector.reduce_sum(
                        out=parts[:, r * NCHUNK + c : r * NCHUNK + c + 1],
                        in_=xt,
                        axis=mybir.AxisListType.X,
                    )
            s2 = small.tile([128, RBLK], F32)
            nc.vector.reduce_sum(
                out=s2,
                in_=parts.rearrange("p (r c) -> p r c", r=RBLK),
                axis=mybir.AxisListType.X,
            )

            # ---- masked lhsT (128, 4): col n nonzero only on its own half ----
            # s2m[(n%2)*64 + ci, n] = S[n, ci], zeros elsewhere
            s2m = small.tile([128, NLOC], F32)
            nc.vector.memset(s2m, 0.0)
            s2m_v = s2m.rearrange("p (r t) -> p r t", t=2)  # [p, r, halfsel]
            s2_v = s2.rearrange("p (r t) -> p r t", t=1)  # [p, r, 1]
            nc.vector.tensor_copy(s2m_v[0:64, :, 0:1], s2_v[0:64, :, :])
            nc.vector.tensor_copy(s2m_v[64:128, :, 1:2], s2_v[64:128, :, :])

            # ---- pooled^T (4, 128) in PSUM: bias matmul + data matmul ----
            pooled = psum_pool.tile([NLOC, COUT], F32, space="PSUM")
            nc.tensor.matmul(
                out=pooled, lhsT=onesb, rhs=biasrows, start=True, stop=False
            )
            nc.tensor.matmul(out=pooled, lhsT=s2m, rhs=wdup, start=False, stop=True)

            # ---- 10 * log(sum_co exp(pooled)) ----
            expt = small.tile([NLOC, COUT], F32)
            sume = small.tile([NLOC, 1], F32)
            nc.scalar.activation(
                out=expt,
                in_=pooled,
                func=mybir.ActivationFunctionType.Exp,
                accum_out=sume,
            )
            logv = small.tile([NLOC, 1], F32)
            nc.scalar.activation(
                out=logv, in_=sume, func=mybir.ActivationFunctionType.Ln
            )
            outv = small.tile([NLOC, 1], F32)
            nc.scalar.mul(out=outv, in_=logv, mul=10.0)
            nc.sync.dma_start(out=y_d, in_=outv)

    nc.compile()
    return nc


def kernel(x, weight, conv_bias, extra_bias):
    x = np.ascontiguousarray(np.asarray(x, dtype=np.float32))
    weight = np.ascontiguousarray(np.asarray(weight, dtype=np.float32))
    conv_bias = np.ascontiguousarray(np.asarray(conv_bias, dtype=np.float32))
    extra_bias = np.ascontiguousarray(np.asarray(extra_bias, dtype=np.float32))
    assert x.shape == (N, CIN, H, W), x.shape
    assert weight.shape == (CIN, COUT, K, K), weight.shape

    if "nc" not in _CACHE:
        _CACHE["nc"] = _build_module()
    nc = _CACHE["nc"]

    w2 = weight.reshape(CIN, COUT * K * K)
    bs2 = np.ascontiguousarray(
        np.stack([conv_bias, extra_bias], axis=0)
    )  # (2, COUT)
    in_maps = []
    for c in range(NCORES):
        xc = x[c * NLOC : (c + 1) * NLOC].reshape(ROWS, HW)
        in_maps.append({"xc": xc, "w": w2, "bs": bs2})

    trace = os.environ.get("BASS_KERNEL_TRACE") == "1"
    res = run_bass_kernel_spmd(
        nc, in_maps, core_ids=list(range(NCORES)), trace=trace
    )
    _CACHE["last_result"] = res
    return np.concatenate([r["y"] for r in res.results], axis=0)



# revision 11
# speedup vs baseline: 1.1814x; 1.1814x over previous
# Trainium2 Bass kernel for: ConvTranspose2d(64->128, k=4, stride=1) -> spatial
# mean -> +biases -> 10*logsumexp over channels.
#
# Math: with full (K-1) output padding, the mean over the ENTIRE conv-transpose
# output spatial extent sees every input pixel through all K*K taps, so
#   pooled[n,co] = (sum_hw x[n,ci,hw]) @ (sum_kk w[ci,co,kk]) / (Ho*Wo) + cb + eb
# exactly. The conv collapses to a spatial sum + a (Cin x Cout) matmul.
#
# Sharding: data-parallel over batch N=32 across 8 cores (4 batches/core),
# params replicated.
#
# v2 changes vs the 32us baseline (trace-driven):
# - x and weight ship as bf16: halves the HBM stream (2.1 MiB/core) AND
#   enables the DVE 2x/4x perf modes for the spatial reduces (the 2e-2
#   rel-err budget dwarfs bf16 rounding; the conv term is ~1e-3 of the
#   logsumexp input anyway). Biases stay f32.
# - x goes as 4 chunk DMAs split across TWO HWDGE rings (sync + scalar):
#   the baseline's 8 dispatches on one ring serialized ~700-1800ns each and
#   left DMA engines idle between chunks.
# - no masked-lhsT shuffle: the two 128-row blocks contract against the
#   weight sums as two small matmuls over partition halves, writing pooled
#   rows in batch order (0,2,1,3); the host unshard undoes the interleave.
# - weight rides first on the scalar ring so its k-sum reduce (DVE) is done
#   long before the last x chunk lands.
# - pre-placed LoadActFuncSet covering BOTH Exp and Ln (after the scalar
#   ring's DMA dispatches) so no ACT_TABLE_LOAD lands on the critical tail.

import os

import numpy as np
import ml_dtypes

import concourse.bacc as bacc
import concourse.bass as bass
import concourse.mybir as mybir
import concourse.tile as tile
from concourse.bass_utils import run_bass_kernel_spmd
from concourse.hw_specs import get_activation_tables

N, CIN, COUT, K, H, W = 32, 64, 128, 4, 64, 64
NCORES = 8
NLOC = N // NCORES          # 4 batches per core
HW = H * W                  # 4096
ROWS = NLOC * CIN           # 256 rows (n,ci) per core
RBLK = ROWS // 128          # 2 row blocks of 128 partitions
CHUNK = 2048                # bf16 column chunk (4 KiB DMA lines, 512 KiB/chunk)
NCHUNK = HW // CHUNK        # 2 chunks per row block
SCALE = 1.0 / float((H + K - 1) * (W + K - 1))   # 1/4489

F32 = mybir.dt.float32
BF16 = mybir.dt.bfloat16
NPBF16 = ml_dtypes.bfloat16

_CACHE: dict = {}


def _build_module() -> bacc.Bacc:
    nc = bacc.Bacc("TRN2", target_bir_lowering=False, enable_partition_id=False)

    x_d = nc.dram_tensor("xc", [ROWS, HW], BF16, kind="ExternalInput").ap()
    w_d = nc.dram_tensor("w", [CIN, COUT * K * K], BF16, kind="ExternalInput").ap()
    bs_d = nc.dram_tensor("bs", [2, COUT], F32, kind="ExternalInput").ap()
    y_d = nc.dram_tensor("y", [NLOC, 1], F32, kind="ExternalOutput").ap()

    with tile.TileContext(nc) as tc:
        with (
            tc.tile_pool(name="xpool", bufs=RBLK * NCHUNK) as xpool,
            tc.tile_pool(name="small", bufs=1) as small,
            tc.tile_pool(name="psum", bufs=1, space="PSUM") as psum_pool,
        ):
            # ---- param + x DMAs (dispatch order == ring order) ----
            biasrows = small.tile([2, COUT], F32)
            nc.scalar.dma_start(out=biasrows, in_=bs_d)
            wk = small.tile([CIN, COUT * K * K], BF16)
            nc.scalar.dma_start(out=wk, in_=w_d)

            xts = []
            for i in range(RBLK * NCHUNK):
                r, c = i // NCHUNK, i % NCHUNK
                xt = xpool.tile([128, CHUNK], BF16, tag="xt")
                eng = nc.sync if c == 0 else nc.scalar
                eng.dma_start(
                    out=xt,
                    in_=x_d[r * 128 : (r + 1) * 128, c * CHUNK : (c + 1) * CHUNK],
                )
                xts.append(xt)

            # one ACT table set covering BOTH Exp and Ln, loaded behind the
            # scalar ring's DMA dispatches, well before the Exp on the tail
            act_tables = get_activation_tables(nc.m.arch)
            set_id = next(
                i
                for i, (_, funcs) in enumerate(act_tables.items())
                if mybir.ActivationFunctionType.Exp in funcs
                and mybir.ActivationFunctionType.Ln in funcs
            )
            nc.scalar.add_instruction(
                mybir.InstLoadActFuncSet(
                    name=nc.get_next_instruction_name(), act_func_set_id=set_id
                )
            )

            # ---- weight k-sums (DVE, done while x streams) ----
            # bf16 outs are fine: the 2048-element spatial accumulations go
            # to f32 `parts`; only the 16-tap k-sum and the 2-chunk combine
            # round to bf16, ~0.4% on a term that is ~1e-3 of the LSE input.
            lowp = lambda: nc.allow_low_precision(
                reason="conv term is tiny vs bias"
            )
            wdup = small.tile([128, COUT], BF16)
            with lowp():
                nc.vector.reduce_sum(
                    out=wdup[0:CIN, :],
                    in_=wk.rearrange("p (c k) -> p c k", k=K * K),
                    axis=mybir.AxisListType.X,
                )
            # fold the 1/(Ho*Wo) mean scale into the weight sums
            nc.vector.tensor_scalar_mul(
                out=wdup[0:CIN, :], in0=wdup[0:CIN, :], scalar1=SCALE
            )
            # PE needs lhsT/rhs on the same base partition: mirror the weight
            # sums onto the upper half (16 KiB on-chip DMA, off critical path)
            nc.sync.dma_start(out=wdup[CIN:128, :], in_=wdup[0:CIN, :])

            onesb = small.tile([2, NLOC], F32)
            nc.vector.memset(onesb, 1.0)

            # ---- spatial sums of x (DVE, per chunk as it lands) ----
            # s2m is the zero-masked (128, 4) lhsT: col n is nonzero only on
            # partition half n%2 (s2m[(n%2)*64 + ci, n] = sum_hw x[n,ci,:]).
            # The combine-reduces write straight into those positions, so no
            # post-stream copy shuffle is needed — just this early memset.
            s2m = small.tile([128, NLOC], BF16)
            nc.vector.memset(s2m, 0.0)
            parts = small.tile([128, RBLK * NCHUNK], F32)
            for i in range(RBLK * NCHUNK):
                nc.vector.reduce_sum(
                    out=parts[:, i : i + 1], in_=xts[i], axis=mybir.AxisListType.X
                )
                if i % NCHUNK == NCHUNK - 1:
                    r = i // NCHUNK
                    pblk = parts[:, r * NCHUNK : (r + 1) * NCHUNK]
                    with lowp():
                        nc.vector.reduce_sum(
                            out=s2m[0:64, 2 * r : 2 * r + 1],
                            in_=pblk[0:64, :],
                            axis=mybir.AxisListType.X,
                        )
                        nc.vector.reduce_sum(
                            out=s2m[64:128, 2 * r + 1 : 2 * r + 2],
                            in_=pblk[64:128, :],
                            axis=mybir.AxisListType.X,
                        )

            # ---- pooled (4, 128) in PSUM: bias matmul + data matmul ----
            pooled = psum_pool.tile([NLOC, COUT], F32, space="PSUM")
            nc.tensor.matmul(
                out=pooled, lhsT=onesb, rhs=biasrows, start=True, stop=False
            )
            nc.tensor.matmul(
                out=pooled, lhsT=s2m, rhs=wdup, start=False, stop=True,
                skip_group_check=True,
            )

            # ---- 10 * log(sum_co exp(pooled)) ----
            expt = small.tile([NLOC, COUT], F32)
            sume = small.tile([NLOC, 1], F32)
            nc.scalar.activation(
                out=expt,
                in_=pooled,
                func=mybir.ActivationFunctionType.Exp,
                accum_out=sume,
            )
            logv = small.tile([NLOC, 1], F32)
            nc.scalar.activation(
                out=logv, in_=sume, func=mybir.ActivationFunctionType.Ln
            )
            outv = small.tile([NLOC, 1], F32)
            nc.vector.tensor_scalar_mul(out=outv, in0=logv, scalar1=10.0)
            nc.sync.dma_start(out=y_d, in_=outv)

    nc.compile()
    return nc


def kernel(x, weight, conv_bias, extra_bias):
    x = np.ascontiguousarray(np.asarray(x, dtype=np.float32))
    weight = np.ascontiguousarray(np.asarray(weight, dtype=np.float32))
    conv_bias = np.ascontiguousarray(np.asarray(conv_bias, dtype=np.float32))
    extra_bias = np.ascontiguousarray(np.asarray(extra_bias, dtype=np.float32))
    assert x.shape == (N, CIN, H, W), x.shape
    assert weight.shape == (CIN, COUT, K, K), weight.shape

    if "nc" not in _CACHE:
        _CACHE["nc"] = _build_module()
    nc = _CACHE["nc"]

    xb = x.reshape(N * CIN, HW).astype(NPBF16)
    w2 = np.ascontiguousarray(weight.reshape(CIN, COUT * K * K).astype(NPBF16))
    bs2 = np.ascontiguousarray(
        np.stack([conv_bias, extra_bias], axis=0)
    )  # (2, COUT)
    in_maps = []
    for c in range(NCORES):
        xc = np.ascontiguousarray(xb[c * ROWS : (c + 1) * ROWS])
        in_maps.append({"xc": xc, "w": w2, "bs": bs2})

    trace = os.environ.get("BASS_KERNEL_TRACE") == "1"
    res = run_bass_kernel_spmd(
        nc, in_maps, core_ids=list(range(NCORES)), trace=trace
    )
    _CACHE["last_result"] = res
    return np.concatenate([r["y"] for r in res.results], axis=0)


# revision 17
# speedup vs baseline: 1.2169x; 1.0301x over previous
# Trainium2 Bass kernel for: ConvTranspose2d(64->128, k=4, stride=1) -> spatial
# mean -> +biases -> 10*logsumexp over channels.
#
# Math: with full (K-1) output padding, the mean over the ENTIRE conv-transpose
# output spatial extent sees every input pixel through all K*K taps, so
#   pooled[n,co] = (sum_hw x[n,ci,hw]) @ (sum_kk w[ci,co,kk]) / (Ho*Wo) + cb + eb
# exactly. The conv collapses to a spatial sum + a (Cin x Cout) matmul.
#
# Sharding: data-parallel over batch N=32 across 8 cores (4 batches/core),
# params replicated.
#
# v3 (trace-driven, from 36us baseline):
# - x/weight ship as bf16 (halves the HBM stream; rel-err budget is 2e-2 and
#   the conv term is ~1e-3 of the logsumexp input, so bf16 rounding is noise).
# - everything streams on ONE HWDGE ring (sync): the DMA engines drain the
#   sync ring completely before serving the scalar ring, so a 2-ring split
#   just serializes (v2 trace); one ring sustains ~378 GB/s.
# - reduces run at 1 elem/lane/cycle everywhere (no DVE 2x mode for reduces),
#   so the spatial sums use DVE tensor_tensor_reduce, which adds TWO chunks
#   elementwise and accumulates in one pass (~0.52 ns/col effective), with
#   ACT (Copy + accumulator) taking the last small chunk in parallel.
# - ACT table set "natural_log_exp_and_others" covers Copy+Exp+Ln; one
#   preload at the top, no reloads on the tail.
# - the tiny combine/epilogue hops (masked copies/adds into the lhsT, x10)
#   sit on Pool, which is otherwise idle.

import os

import numpy as np
import ml_dtypes

import concourse.bacc as bacc
import concourse.bass as bass
import concourse.mybir as mybir
import concourse.tile as tile
from concourse.bass_utils import run_bass_kernel_spmd
from concourse.hw_specs import get_activation_tables

N, CIN, COUT, K, H, W = 32, 64, 128, 4, 64, 64
NCORES = 8
NLOC = N // NCORES          # 4 batches per core
HW = H * W                  # 4096
ROWS = NLOC * CIN           # 256 rows (n,ci) per core
RBLK = ROWS // 128          # 2 row blocks of 128 partitions
SCALE = 1.0 / float((H + K - 1) * (W + K - 1))   # 1/4489

F32 = mybir.dt.float32
BF16 = mybir.dt.bfloat16
NPBF16 = ml_dtypes.bfloat16

# x split: block0 = one DVE ttr pair; block1 = DVE ttr pair + ACT tail chunk
W0 = 2048            # block0 ttr half-width (cols 0:2048 + 2048:4096)
W1 = 1536            # block1 ttr half-width (cols 0:1536 + 1536:3072)
WD = HW - 2 * W1     # 1024: block1 ACT tail chunk (cols 3072:4096)

_CACHE: dict = {}

# bisect flags (dev only; final config hardcodes these)
USE_TTR = os.environ.get("K_TTR", "1") == "1"
USE_GPSIMD = os.environ.get("K_GPS", "1") == "1"
USE_ACT_RED = os.environ.get("K_ACT", "1") == "1"


def _build_module() -> bacc.Bacc:
    nc = bacc.Bacc("TRN2", target_bir_lowering=False, enable_partition_id=False)

    x_d = nc.dram_tensor("xc", [ROWS, HW], BF16, kind="ExternalInput").ap()
    w_d = nc.dram_tensor("w", [CIN, COUT * K * K], BF16, kind="ExternalInput").ap()
    bs_d = nc.dram_tensor("bs", [2, COUT], F32, kind="ExternalInput").ap()
    y_d = nc.dram_tensor("y", [NLOC, 1], F32, kind="ExternalOutput").ap()

    ADD = mybir.AluOpType.add

    with tile.TileContext(nc) as tc:
        with (
            tc.tile_pool(name="xpool", bufs=5) as xpool,
            tc.tile_pool(name="small", bufs=1) as small,
            tc.tile_pool(name="psum", bufs=1, space="PSUM") as psum_pool,
        ):
            # one ACT table set covering Copy AND Exp AND Ln, loaded once
            act_tables = get_activation_tables(nc.m.arch)
            set_id = next(
                i
                for i, (_, funcs) in enumerate(act_tables.items())
                if mybir.ActivationFunctionType.Exp in funcs
                and mybir.ActivationFunctionType.Ln in funcs
                and mybir.ActivationFunctionType.Copy in funcs
            )
            nc.scalar.add_instruction(
                mybir.InstLoadActFuncSet(
                    name=nc.get_next_instruction_name(), act_func_set_id=set_id
                )
            )

            lowp = lambda: nc.allow_low_precision(
                reason="conv term is tiny vs bias"
            )

            # ---- everything on the sync HWDGE ring, arrival order ----
            biasrows = small.tile([2, COUT], F32)
            nc.sync.dma_start(out=biasrows, in_=bs_d)
            wk = small.tile([CIN, COUT * K * K], BF16)
            nc.sync.dma_start(out=wk, in_=w_d)

            def ld(rb, lo, hi):
                xt = xpool.tile([128, hi - lo], BF16, tag="xt")
                nc.sync.dma_start(
                    out=xt, in_=x_d[rb * 128 : (rb + 1) * 128, lo:hi]
                )
                return xt

            a0 = ld(0, 0, W0)
            b0 = ld(0, W0, 2 * W0)
            a1 = ld(1, 0, W1)
            b1 = ld(1, W1, 2 * W1)
            d1 = ld(1, 2 * W1, HW)

            # ---- weight k-sums (DVE) + mirror to upper partitions ----
            wdup = small.tile([128, COUT], BF16)
            with lowp():
                nc.vector.reduce_sum(
                    out=wdup[0:CIN, :],
                    in_=wk.rearrange("p (c k) -> p c k", k=K * K),
                    axis=mybir.AxisListType.X,
                )
            # fold the 1/(Ho*Wo) mean scale into the weight sums
            nc.vector.tensor_scalar_mul(
                out=wdup[0:CIN, :], in0=wdup[0:CIN, :], scalar1=SCALE
            )
            # PE needs lhsT/rhs on the same base partition: mirror the weight
            # sums onto the upper half (16 KiB on-chip DMA, off critical path)
            nc.sync.dma_start(out=wdup[CIN:128, :], in_=wdup[0:CIN, :])

            cheap = nc.gpsimd if USE_GPSIMD else nc.vector
            onesb = small.tile([2, NLOC], F32)
            cheap.memset(onesb, 1.0)
            # s2m is the zero-masked (128, 4) lhsT: col n nonzero only on
            # partition half n%2 (s2m[(n%2)*64 + ci, n] = sum_hw x[n,ci,:]).
            s2m = small.tile([128, NLOC], BF16)
            cheap.memset(s2m, 0.0)

            # ---- spatial sums: DVE fused add+accum pairs, ACT tail ----
            parts = small.tile([128, 3], F32)
            scr = small.tile([128, W0], BF16)
            if USE_TTR:
                nc.vector.tensor_tensor_reduce(
                    out=scr, in0=a0, in1=b0, scale=1.0, scalar=0.0,
                    op0=ADD, op1=ADD, accum_out=parts[:, 0:1],
                )
                nc.vector.tensor_tensor_reduce(
                    out=scr[:, 0:W1], in0=a1, in1=b1, scale=1.0, scalar=0.0,
                    op0=ADD, op1=ADD, accum_out=parts[:, 1:2],
                )
            else:
                p01 = small.tile([128, 4], F32)
                for j, t in enumerate([a0, b0, a1, b1]):
                    nc.vector.reduce_sum(
                        out=p01[:, j : j + 1], in_=t, axis=mybir.AxisListType.X
                    )
                nc.vector.reduce_sum(
                    out=parts[:, 0:1],
                    in_=p01[:, 0:2],
                    axis=mybir.AxisListType.X,
                )
                nc.vector.reduce_sum(
                    out=parts[:, 1:2],
                    in_=p01[:, 2:4],
                    axis=mybir.AxisListType.X,
                )
            if USE_ACT_RED:
                dscr = small.tile([128, WD], BF16)
                nc.scalar.activation(
                    out=dscr,
                    in_=d1,
                    func=mybir.ActivationFunctionType.Copy,
                    accum_out=parts[:, 2:3],
                )
            else:
                nc.vector.reduce_sum(
                    out=parts[:, 2:3], in_=d1, axis=mybir.AxisListType.X
                )

            # masked combines into s2m on Pool (idle at the tail)
            with lowp():
                cheap.tensor_copy(s2m[0:64, 0:1], parts[0:64, 0:1])
                cheap.tensor_copy(s2m[64:128, 1:2], parts[64:128, 0:1])
                cheap.tensor_add(
                    s2m[0:64, 2:3], parts[0:64, 1:2], parts[0:64, 2:3]
                )
                cheap.tensor_add(
                    s2m[64:128, 3:4], parts[64:128, 1:2], parts[64:128, 2:3]
                )

            # ---- pooled (4, 128) in PSUM: bias matmul + data matmul ----
            pooled = psum_pool.tile([NLOC, COUT], F32, space="PSUM")
            nc.tensor.matmul(
                out=pooled, lhsT=onesb, rhs=biasrows, start=True, stop=False
            )
            nc.tensor.matmul(
                out=pooled, lhsT=s2m, rhs=wdup, start=False, stop=True
            )

            # ---- 10 * log(sum_co exp(pooled)) ----
            expt = small.tile([NLOC, COUT], F32)
            sume = small.tile([NLOC, 1], F32)
            nc.scalar.activation(
                out=expt,
                in_=pooled,
                func=mybir.ActivationFunctionType.Exp,
                accum_out=sume,
            )
            logv = small.tile([NLOC, 1], F32)
            nc.scalar.activation(
                out=logv, in_=sume, func=mybir.ActivationFunctionType.Ln
            )
            outv = small.tile([NLOC, 1], F32)
            cheap.tensor_scalar_mul(out=outv, in0=logv, scalar1=10.0)
            nc.sync.dma_start(out=y_d, in_=outv)

    nc.compile()
    return nc


def kernel(x, weight, conv_bias, extra_bias):
    x = np.ascontiguousarray(np.asarray(x, dtype=np.float32))
    weight = np.ascontiguousarray(np.asarray(weight, dtype=np.float32))
    conv_bias = np.ascontiguousarray(np.asarray(conv_bias, dtype=np.float32))
    extra_bias = np.ascontiguousarray(np.asarray(extra_bias, dtype=np.float32))
    assert x.shape == (N, CIN, H, W), x.shape
    assert weight.shape == (CIN, COUT, K, K), weight.shape

    if "nc" not in _CACHE:
        _CACHE["nc"] = _build_module()
    nc = _CACHE["nc"]

    xb = x.reshape(N * CIN, HW).astype(NPBF16)
    w2 = np.ascontiguousarray(weight.reshape(CIN, COUT * K * K).astype(NPBF16))
    bs2 = np.ascontiguousarray(
        np.stack([conv_bias, extra_bias], axis=0)
    )  # (2, COUT)
    in_maps = []
    for c in range(NCORES):
        xc = np.ascontiguousarray(xb[c * ROWS : (c + 1) * ROWS])
        in_maps.append({"xc": xc, "w": w2, "bs": bs2})

    trace = os.environ.get("BASS_KERNEL_TRACE") == "1"
    res = run_bass_kernel_spmd(
        nc, in_maps, core_ids=list(range(NCORES)), trace=trace
    )
    _CACHE["last_result"] = res
    return np.concatenate([r["y"] for r in res.results], axis=0)


# revision 18
# speedup vs baseline: 1.2710x; 1.0444x over previous
# Trainium2 Bass kernel for: ConvTranspose2d(64->128, k=4, stride=1) -> spatial
# mean -> +biases -> 10*logsumexp over channels.
#
# Math: with full (K-1) output padding, the mean over the ENTIRE conv-transpose
# output spatial extent sees every input pixel through all K*K taps, so
#   pooled[n,co] = (sum_hw x[n,ci,hw]) @ (sum_kk w[ci,co,kk]) / (Ho*Wo) + cb + eb
# exactly. The conv collapses to a spatial sum + a (Cin x Cout) matmul.
#
# Sharding: data-parallel over batch N=32 across 8 cores (4 batches/core),
# params replicated.
#
# v3 (trace-driven, from 36us baseline):
# - x/weight ship as bf16 (halves the HBM stream; rel-err budget is 2e-2 and
#   the conv term is ~1e-3 of the logsumexp input, so bf16 rounding is noise).
# - everything streams on ONE HWDGE ring (sync): the DMA engines drain the
#   sync ring completely before serving the scalar ring, so a 2-ring split
#   just serializes (v2 trace); one ring sustains ~378 GB/s.
# - reduces run at 1 elem/lane/cycle everywhere (no DVE 2x mode for reduces),
#   so the spatial sums use DVE tensor_tensor_reduce, which adds TWO chunks
#   elementwise and accumulates in one pass (~0.52 ns/col effective), with
#   ACT (Copy + accumulator) taking the last small chunk in parallel.
# - ACT table set "natural_log_exp_and_others" covers Copy+Exp+Ln; one
#   preload at the top, no reloads on the tail.
# - the tiny combine/epilogue hops (masked copies/adds into the lhsT, x10)
#   sit on Pool, which is otherwise idle.

import os

import numpy as np
import ml_dtypes

import concourse.bacc as bacc
import concourse.bass as bass
import concourse.mybir as mybir
import concourse.tile as tile
from concourse.bass_utils import run_bass_kernel_spmd
from concourse.hw_specs import get_activation_tables

N, CIN, COUT, K, H, W = 32, 64, 128, 4, 64, 64
NCORES = 8
NLOC = N // NCORES          # 4 batches per core
HW = H * W                  # 4096
ROWS = NLOC * CIN           # 256 rows (n,ci) per core
RBLK = ROWS // 128          # 2 row blocks of 128 partitions
SCALE = 1.0 / float((H + K - 1) * (W + K - 1))   # 1/4489

F32 = mybir.dt.float32
BF16 = mybir.dt.bfloat16
NPBF16 = ml_dtypes.bfloat16

# x split: block0 = one DVE ttr pair; block1 = DVE ttr pair + ACT tail chunk
W0 = 2048            # block0 ttr half-width (cols 0:2048 + 2048:4096)
W1 = 1536            # block1 ttr half-width (cols 0:1536 + 1536:3072)
WD = HW - 2 * W1     # 1024: block1 ACT tail chunk (cols 3072:4096)

_CACHE: dict = {}

# bisect flags (dev only; final config hardcodes these)
USE_TTR = os.environ.get("K_TTR", "1") == "1"
USE_GPSIMD = os.environ.get("K_GPS", "1") == "1"
USE_ACT_RED = os.environ.get("K_ACT", "1") == "1"


def _build_module() -> bacc.Bacc:
    nc = bacc.Bacc("TRN2", target_bir_lowering=False, enable_partition_id=False)

    x_d = nc.dram_tensor("xc", [ROWS, HW], BF16, kind="ExternalInput").ap()
    w_d = nc.dram_tensor("w", [CIN, COUT * K * K], BF16, kind="ExternalInput").ap()
    bs_d = nc.dram_tensor("bs", [2, COUT], F32, kind="ExternalInput").ap()
    y_d = nc.dram_tensor("y", [NLOC, 1], F32, kind="ExternalOutput").ap()

    ADD = mybir.AluOpType.add

    with tile.TileContext(nc) as tc:
        with (
            tc.tile_pool(name="xpool", bufs=5) as xpool,
            tc.tile_pool(name="small", bufs=1) as small,
            tc.tile_pool(name="psum", bufs=1, space="PSUM") as psum_pool,
        ):
            # one ACT table set covering Copy AND Exp AND Ln, loaded once
            act_tables = get_activation_tables(nc.m.arch)
            set_id = next(
                i
                for i, (_, funcs) in enumerate(act_tables.items())
                if mybir.ActivationFunctionType.Exp in funcs
                and mybir.ActivationFunctionType.Ln in funcs
                and mybir.ActivationFunctionType.Copy in funcs
            )
            nc.scalar.add_instruction(
                mybir.InstLoadActFuncSet(
                    name=nc.get_next_instruction_name(), act_func_set_id=set_id
                )
            )

            lowp = lambda: nc.allow_low_precision(
                reason="conv term is tiny vs bias"
            )

            # ---- everything on the sync HWDGE ring, arrival order ----
            biasrows = small.tile([2, COUT], F32)
            nc.sync.dma_start(out=biasrows, in_=bs_d)
            wk = small.tile([CIN, COUT * K * K], BF16)
            nc.sync.dma_start(out=wk, in_=w_d)

            def ld(rb, lo, hi):
                xt = xpool.tile([128, hi - lo], BF16, tag="xt")
                nc.sync.dma_start(
                    out=xt, in_=x_d[rb * 128 : (rb + 1) * 128, lo:hi]
                )
                return xt

            a0 = ld(0, 0, W0)
            b0 = ld(0, W0, 2 * W0)
            a1 = ld(1, 0, W1)
            b1 = ld(1, W1, 2 * W1)
            d1 = ld(1, 2 * W1, HW)

            # ---- weight k-sums (DVE) + mirror to upper partitions ----
            wdup = small.tile([128, COUT], BF16)
            with lowp():
                nc.vector.reduce_sum(
                    out=wdup[0:CIN, :],
                    in_=wk.rearrange("p (c k) -> p c k", k=K * K),
                    axis=mybir.AxisListType.X,
                )
            # fold the 1/(Ho*Wo) mean scale into the weight sums
            nc.vector.tensor_scalar_mul(
                out=wdup[0:CIN, :], in0=wdup[0:CIN, :], scalar1=SCALE
            )
            # PE needs lhsT/rhs on the same base partition: mirror the weight
            # sums onto the upper half (16 KiB on-chip DMA, off critical path)
            nc.sync.dma_start(out=wdup[CIN:128, :], in_=wdup[0:CIN, :])

            cheap = nc.gpsimd if USE_GPSIMD else nc.vector
            onesb = small.tile([2, NLOC], F32)
            cheap.memset(onesb, 1.0)
            # s2m is the zero-masked (128, 4) lhsT: col n nonzero only on
            # partition half n%2 (s2m[(n%2)*64 + ci, n] = sum_hw x[n,ci,:]).
            s2m = small.tile([128, NLOC], BF16)
            cheap.memset(s2m, 0.0)

            # ---- spatial sums: DVE fused add+accum pairs, ACT tail ----
            parts = small.tile([128, 3], F32)
            scr = small.tile([128, W0], BF16)
            if USE_TTR:
                # fused (a + b) with free-axis accumulator: one pass over
                # two chunks (InstTensorScalarPtr encoding; the dedicated
                # InstTensorTensorReduce dies on real hardware)
                nc.vector.scalar_tensor_tensor(
                    out=scr, in0=a0, scalar=1.0, in1=b0,
                    op0=mybir.AluOpType.mult, op1=ADD,
                    accum_out=parts[:, 0:1],
                )
                nc.vector.scalar_tensor_tensor(
                    out=scr[:, 0:W1], in0=a1, scalar=1.0, in1=b1,
                    op0=mybir.AluOpType.mult, op1=ADD,
                    accum_out=parts[:, 1:2],
                )
            else:
                p01 = small.tile([128, 4], F32)
                for j, t in enumerate([a0, b0, a1, b1]):
                    nc.vector.reduce_sum(
                        out=p01[:, j : j + 1], in_=t, axis=mybir.AxisListType.X
                    )
                nc.vector.reduce_sum(
                    out=parts[:, 0:1],
                    in_=p01[:, 0:2],
                    axis=mybir.AxisListType.X,
                )
                nc.vector.reduce_sum(
                    out=parts[:, 1:2],
                    in_=p01[:, 2:4],
                    axis=mybir.AxisListType.X,
                )
            if USE_ACT_RED:
                dscr = small.tile([128, WD], BF16)
                nc.scalar.activation(
                    out=dscr,
                    in_=d1,
                    func=mybir.ActivationFunctionType.Copy,
                    accum_out=parts[:, 2:3],
                )
            else:
                nc.vector.reduce_sum(
                    out=parts[:, 2:3], in_=d1, axis=mybir.AxisListType.X
                )

            # masked combines into s2m on Pool (idle at the tail)
            with lowp():
                cheap.tensor_copy(s2m[0:64, 0:1], parts[0:64, 0:1])
                cheap.tensor_copy(s2m[64:128, 1:2], parts[64:128, 0:1])
                cheap.tensor_add(
                    s2m[0:64, 2:3], parts[0:64, 1:2], parts[0:64, 2:3]
                )
                cheap.tensor_add(
                    s2m[64:128, 3:4], parts[64:128, 1:2], parts[64:128, 2:3]
                )

            # ---- pooled (4, 128) in PSUM: bias matmul + data matmul ----
            pooled = psum_pool.tile([NLOC, COUT], F32, space="PSUM")
            nc.tensor.matmul(
                out=pooled, lhsT=onesb, rhs=biasrows, start=True, stop=False
            )
            nc.tensor.matmul(
                out=pooled, lhsT=s2m, rhs=wdup, start=False, stop=True
            )

            # ---- 10 * log(sum_co exp(pooled)) ----
            expt = small.tile([NLOC, COUT], F32)
            sume = small.tile([NLOC, 1], F32)
            nc.scalar.activation(
                out=expt,
                in_=pooled,
                func=mybir.ActivationFunctionType.Exp,
                accum_out=sume,
            )
            logv = small.tile([NLOC, 1], F32)
            nc.scalar.activation(
                out=logv, in_=sume, func=mybir.ActivationFunctionType.Ln
            )
            outv = small.tile([NLOC, 1], F32)
            cheap.tensor_scalar_mul(out=outv, in0=logv, scalar1=10.0)
            nc.sync.dma_start(out=y_d, in_=outv)

    nc.compile()
    return nc


def kernel(x, weight, conv_bias, extra_bias):
    x = np.ascontiguousarray(np.asarray(x, dtype=np.float32))
    weight = np.ascontiguousarray(np.asarray(weight, dtype=np.float32))
    conv_bias = np.ascontiguousarray(np.asarray(conv_bias, dtype=np.float32))
    extra_bias = np.ascontiguousarray(np.asarray(extra_bias, dtype=np.float32))
    assert x.shape == (N, CIN, H, W), x.shape
    assert weight.shape == (CIN, COUT, K, K), weight.shape

    if "nc" not in _CACHE:
        _CACHE["nc"] = _build_module()
    nc = _CACHE["nc"]

    xb = x.reshape(N * CIN, HW).astype(NPBF16)
    w2 = np.ascontiguousarray(weight.reshape(CIN, COUT * K * K).astype(NPBF16))
    bs2 = np.ascontiguousarray(
        np.stack([conv_bias, extra_bias], axis=0)
    )  # (2, COUT)
    in_maps = []
    for c in range(NCORES):
        xc = np.ascontiguousarray(xb[c * ROWS : (c + 1) * ROWS])
        in_maps.append({"xc": xc, "w": w2, "bs": bs2})

    trace = os.environ.get("BASS_KERNEL_TRACE") == "1"
    res = run_bass_kernel_spmd(
        nc, in_maps, core_ids=list(range(NCORES)), trace=trace
    )
    _CACHE["last_result"] = res
    return np.concatenate([r["y"] for r in res.results], axis=0)
